# revision 19
# baseline (speedup 1.0000x reference)
"""MinGRU layer (LN -> gate/candidate Linear -> minGRU scan -> residual) on 8 trn2 cores.

Problem (hardcoded): x [B=4, T=4096, H=1024] fp32, weights Wg/Wc [1024,1024],
biases bg/bc [1024], LN gamma/beta [1024].

Sharding: core c = (batch b = c//2, output-half p = c%2). Each core receives the
full transposed batch row (H on partitions, T on free) and computes z/c for its
512 output channels over all T; the minGRU recurrence is elementwise over (b,h)
so no collectives are needed.

Per 512-col chunk, built around fp8 DoubleRow matmuls (each streams a PAIR of
k-tiles -> 2x bf16 GEMM throughput; weights pre-scaled by 32 on host so fp8
stays in normal range):
  1. GEMM on RAW fp8 x (no prescale): P' = 32W.x8 over 4 k-pair DR matmuls plus
     one rank-1 aug DR matmul carrying both LayerNorm corrections:
     slot0 = (-4*wsumq) x (8*mu), slot1 = (32*b_eff) x (1/rstd).  Then
     tmp = P' * (rstd/32 broadcast) = W.((x-mu)*rstd) + b exactly (post-scale
     keeps P' in fp32 PSUM - no fp8 requantize of the scaled activations).
  2. Stats by the same DR trick: ones-weights (col 0 / col 1) sum x8 and x8^2
     pairs into rows 0/1 of one PSUM bank; squares x8^2 on ScalarE.
  3. rstd via 2 Newton steps for 1/sqrt(var+eps) on GpSimd in a DMA-transposed
     [128, 2, 4] layout (t on partitions, both chunks of a pair at once) - no
     Ln/Exp, so the ACT table never leaves the sigmoid/square/copy set.
  4. z = sigmoid(tmp_g) on ScalarE (bias pre-folded via aug slot1); zbar = 1-z
     (VectorE 4x); bsc = tmp_c*z (GpSimd); h = scan(zbar, bsc) (VectorE, the
     only engine with tensor_tensor_scan); residual h + x(fp16) on VectorE in
     2x mode; fp16 output, host transposes back.
"""

import os
import numpy as np
import ml_dtypes

import concourse.bass as bass
import concourse.bacc as bacc
import concourse.tile as tile
from concourse import mybir
from concourse.bass_utils import run_bass_kernel_spmd

B, T, H = 4, 4096, 1024
EPS = 1e-5
N_CORES = 8
OH = H // 2          # output channels per core
CHUNK = 512
N_CHUNKS = T // CHUNK
KT = H // 128        # k-tiles (contraction)
OT = OH // 128       # o-tiles per core
NPAIR = N_CHUNKS // 2

F32 = mybir.dt.float32
BF16 = mybir.dt.bfloat16
FP16 = mybir.dt.float16
FP8 = mybir.dt.float8e4
AF = mybir.ActivationFunctionType
OP = mybir.AluOpType
DR = mybir.MatmulPerfMode.DoubleRow
BF = ml_dtypes.bfloat16
F8 = ml_dtypes.float8_e4m3

_CACHE = {}


def _build():
    nc = bacc.Bacc("TRN2", target_bir_lowering=False, debug=False)

    x8_d = nc.dram_tensor("x8", [N_CHUNKS, 128, KT, CHUNK], FP8, kind="ExternalInput").ap()
    xr_d = nc.dram_tensor("xr", [N_CHUNKS, 128, OT, CHUNK], FP16, kind="ExternalInput").ap()
    wg_d = nc.dram_tensor("wg", [128, KT // 2, OT, 2, 128], FP8, kind="ExternalInput").ap()
    wc_d = nc.dram_tensor("wc", [128, KT // 2, OT, 2, 128], FP8, kind="ExternalInput").ap()
    augg_d = nc.dram_tensor("augg", [1, OT, 2, 128], FP8, kind="ExternalInput").ap()
    augc_d = nc.dram_tensor("augc", [1, OT, 2, 128], FP8, kind="ExternalInput").ap()
    onx_d = nc.dram_tensor("onx", [128, 2, 128], FP8, kind="ExternalInput").ap()
    onq_d = nc.dram_tensor("onq", [128, 2, 128], FP8, kind="ExternalInput").ap()
    onr_d = nc.dram_tensor("onr", [1, 128], BF16, kind="ExternalInput").ap()
    out_d = nc.dram_tensor("outT", [N_CHUNKS, OT, 128, CHUNK], FP16, kind="ExternalOutput").ap()

    # Newton scratch in DRAM (partition-crossing transposes go through HBM).
    # stg rows per pair: (sx0, sq0, sx1, sq1)
    st_dram = nc.dram_tensor("st_sc", [NPAIR, 4, CHUNK], F32, kind="ExternalOutput").ap()
    rs_dram = nc.dram_tensor("rs_sc", [NPAIR, 2, CHUNK], BF16, kind="ExternalOutput").ap()
    ag_dram = nc.dram_tensor("ag_sc", [NPAIR, 2, 2, CHUNK], FP8, kind="ExternalOutput").ap()

    with tile.TileContext(nc) as tc:
        with (
            tc.tile_pool(name="const", bufs=1) as cpool,
            tc.tile_pool(name="xin", bufs=5) as xpool,
            tc.tile_pool(name="sq", bufs=5) as sqpool,
            tc.tile_pool(name="xr", bufs=3) as xrpool,
            tc.tile_pool(name="nt", bufs=2) as ntpool,
            tc.tile_pool(name="row", bufs=4) as rpool,
            tc.tile_pool(name="work", bufs=2) as wpool,
            tc.tile_pool(name="hbuf", bufs=2) as hpool,
            tc.tile_pool(name="psG", bufs=2, space="PSUM") as psG,
            tc.tile_pool(name="psC", bufs=1, space="PSUM") as psC,
            tc.tile_pool(name="psS", bufs=1, space="PSUM") as psS,
            tc.tile_pool(name="psb", bufs=1, space="PSUM") as psbp,
        ):
            wg_sb = cpool.tile([128, KT // 2, OT, 2, 128], FP8, tag="wg")
            wc_sb = cpool.tile([128, KT // 2, OT, 2, 128], FP8, tag="wc")
            augg = cpool.tile([1, OT, 2, 128], FP8, tag="augg")
            augc = cpool.tile([1, OT, 2, 128], FP8, tag="augc")
            onx = cpool.tile([128, 2, 128], FP8, tag="onx")
            onq = cpool.tile([128, 2, 128], FP8, tag="onq")
            onr = cpool.tile([1, 128], BF16, tag="onr")

            def load_consts():
                nc.scalar.dma_start(onx[:], onx_d)
                nc.scalar.dma_start(onq[:], onq_d)
                nc.scalar.dma_start(onr[:], onr_d)
                nc.scalar.dma_start(wg_sb[:], wg_d)
                nc.scalar.dma_start(wc_sb[:], wc_d)
                nc.scalar.dma_start(augg[:], augg_d)
                nc.scalar.dma_start(augc[:], augc_d)

            x8_t = [None] * N_CHUNKS
            xsq_t = [None] * N_CHUNKS
            xr_t = [None] * N_CHUNKS
            st_t = [None] * N_CHUNKS     # PSUM stats tiles
            rstd_t = [None] * N_CHUNKS   # [1,512] bf16 rows
            augr_t = [None] * N_CHUNKS   # [1,2,512] fp8 rows
            h_prev = [None] * OT

            def load_x(i, split=False):
                xc = xpool.tile([128, KT, CHUNK], FP8, tag="x8")
                if split:
                    half = KT // 2
                    nc.sync.dma_start(xc[:, :half, :], x8_d[i, :, :half, :])
                    nc.sync.dma_start(xc[:, half:, :], x8_d[i, :, half:, :])
                else:
                    nc.sync.dma_start(xc[:], x8_d[i])
                x8_t[i] = xc

            def load_xr(i):
                xr = xrpool.tile([128, OT, CHUNK], FP16, tag="xr")
                nc.sync.dma_start(xr[:], xr_d[i])
                xr_t[i] = xr

            def squares(i):
                xc = x8_t[i]
                sq = sqpool.tile([128, KT, CHUNK], FP8, tag="xsq")
                with nc.allow_low_precision(reason="fp8 squares only feed the var sum"):
                    half = KT // 2
                    nc.scalar.activation(sq[:, :half, :], xc[:, :half, :], AF.Square)
                    nc.scalar.activation(sq[:, half:, :], xc[:, half:, :], AF.Square)
                xsq_t[i] = sq

            def stats(i):
                """st row0 = sum_h x8, row1 = sum_h x8^2 (one PSUM bank)."""
                xc, sq = x8_t[i], xsq_t[i]
                st = psS.tile([128, CHUNK], F32, tag="st")
                for kp in range(KT // 2):
                    nc.tensor.matmul(
                        st[:], onx[:], xc[:, 2 * kp : 2 * kp + 2, :],
                        start=(kp == 0), stop=False, perf_mode=DR,
                    )
                for kp in range(KT // 2):
                    nc.tensor.matmul(
                        st[:], onq[:], sq[:, 2 * kp : 2 * kp + 2, :],
                        start=False, stop=(kp == KT // 2 - 1), perf_mode=DR,
                    )
                st_t[i] = st

            def stg_copy(i):
                # PSUM stats rows -> SBUF staging -> DRAM scratch
                pair, par = divmod(i, 2)
                stg = ntpool.tile([2, CHUNK], F32, tag="stg")
                nc.scalar.activation(stg[:], st_t[i][0:2, :], AF.Copy)
                nc.gpsimd.dma_start(st_dram[pair, 2 * par : 2 * par + 2], stg[:])

            def newton(pair):
                """rstd = 1/sqrt(var+eps) via 2 Newton steps, t-on-partitions."""
                # [4,512] rows (a=chunk, two=kind) -> two loads of [128, a, 4]
                src3 = st_dram[pair].rearrange("(a two) (p j) -> two p a j", two=2, p=128)
                sxT = ntpool.tile([128, 2, 4], F32, tag="sxT")
                sqT = ntpool.tile([128, 2, 4], F32, tag="sqT")
                nc.gpsimd.dma_start(sxT[:], src3[0])
                nc.gpsimd.dma_start(sqT[:], src3[1])
                sx = sxT[:]
                sq = sqT[:]
                mu = ntpool.tile([128, 2, 4], F32, tag="mu")
                m2 = ntpool.tile([128, 2, 4], F32, tag="m2")
                v = ntpool.tile([128, 2, 4], F32, tag="v")
                y = ntpool.tile([128, 2, 4], F32, tag="y")
                t1 = ntpool.tile([128, 2, 4], F32, tag="t1")
                g = nc.gpsimd
                g.tensor_scalar_mul(mu[:], sx, 1.0 / H)
                g.tensor_mul(m2[:], mu[:], mu[:])         # mu^2
                g.tensor_scalar(t1[:], sq, 1.0 / H, EPS, OP.mult, OP.add)
                g.tensor_sub(v[:], t1[:], m2[:])          # var + eps
                g.tensor_scalar(y[:], v[:], -0.5, 1.5, OP.mult, OP.add)
                for _ in range(2):
                    g.tensor_mul(t1[:], v[:], y[:])
                    g.tensor_mul(t1[:], t1[:], y[:])
                    g.tensor_scalar(t1[:], t1[:], -0.5, 1.5, OP.mult, OP.add)
                    g.tensor_mul(y[:], y[:], t1[:])
                rT = ntpool.tile([128, 2, 4], BF16, tag="rT")
                m8 = ntpool.tile([128, 2, 4], FP8, tag="m8")
                i8 = ntpool.tile([128, 2, 4], FP8, tag="i8")
                with nc.allow_low_precision(reason="rstd bf16 / aug rows fp8"):
                    g.tensor_scalar_mul(rT[:], y[:], 1.0)
                    g.tensor_scalar_mul(m8[:], mu[:], 8.0)
                    g.tensor_mul(i8[:], v[:], y[:])       # 1/rstd = (var+eps)*rstd
                nc.gpsimd.dma_start(
                    rs_dram[pair].rearrange("a (p j) -> p a j", p=128), rT[:]
                )
                agr = ag_dram[pair].rearrange("a s (p j) -> s p a j", p=128)
                nc.gpsimd.dma_start(agr[0], m8[:])
                nc.gpsimd.dma_start(agr[1], i8[:])
                for par in range(2):
                    i = 2 * pair + par
                    rr = rpool.tile([1, CHUNK], BF16, tag="rstd")
                    nc.gpsimd.dma_start(rr[:], rs_dram[pair, par : par + 1, :])
                    rstd_t[i] = rr
                    ar = rpool.tile([1, 2, CHUNK], FP8, tag="augr")
                    nc.gpsimd.dma_start(ar[:], ag_dram[pair, par].unsqueeze(0))
                    augr_t[i] = ar

            def gemm_main(i, half):
                """The 16 rstd-independent k-pair matmuls of one o-half."""
                xc = x8_t[i]
                pg = psG.tile([128, 2, CHUNK], F32, tag="pg")
                pc = psC.tile([128, 2, CHUNK], F32, tag="pc")
                for w_sb, dst in ((wg_sb, pg), (wc_sb, pc)):
                    for s in range(2):
                        o = 2 * half + s
                        for kp in range(KT // 2):
                            nc.tensor.matmul(
                                dst[:, s, :],
                                w_sb[:, kp, o],
                                xc[:, 2 * kp : 2 * kp + 2, :],
                                start=(kp == 0), stop=False, perf_mode=DR,
                                skip_group_check=True,
                            )
                return pg, pc

            def gemm_aug(i, half, pg, pc):
                """Rank-1 LN-correction matmuls (need mu/invrstd rows)."""
                ar = augr_t[i]
                for aug, dst in ((augg, pg), (augc, pc)):
                    for s in range(2):
                        o = 2 * half + s
                        nc.tensor.matmul(
                            dst[:, s, :], aug[:, o], ar[:],
                            start=False, stop=True, perf_mode=DR,
                            skip_group_check=True,
                        )

            def chunk_body(i, pre=None):
                if i + 3 < N_CHUNKS:
                    load_x(i + 3)
                if i + 1 < N_CHUNKS:
                    load_xr(i + 1)

                pg0, pc0 = pre[0] if pre else gemm_main(i, 0)
                # rstd-dependent PE work sits after 16 free-running matmuls
                psb = psbp.tile([128, CHUNK], F32, tag="psb")
                nc.tensor.matmul(psb[:], onr[:], rstd_t[i][:], start=True, stop=True)
                gemm_aug(i, 0, pg0, pc0)
                psbS = wpool.tile([128, CHUNK], BF16, tag="psbS")
                with nc.allow_low_precision(reason="bf16 rstd broadcast"):
                    nc.scalar.activation(psbS[:], psb[:], AF.Copy)
                if i + 3 < N_CHUNKS:
                    squares(i + 3)      # ACT fills while PE runs GEMMs
                pg1, pc1 = pre[1] if pre else gemm_main(i, 1)
                if i + 3 < N_CHUNKS:
                    stats(i + 3)        # PE mid-chunk
                gemm_aug(i, 1, pg1, pc1)

                with nc.allow_low_precision(reason="bf16 gate/candidate path"):
                    tg = wpool.tile([128, OT, CHUNK], BF16, tag="tg")
                    tc_ = wpool.tile([128, OT, CHUNK], BF16, tag="tcn")
                    z = wpool.tile([128, OT, CHUNK], BF16, tag="z")
                    zb = wpool.tile([128, OT, CHUNK], BF16, tag="zb")
                    bsc = wpool.tile([128, OT, CHUNK], BF16, tag="bsc")
                    h = hpool.tile([128, OT, CHUNK], BF16, tag="h")
                    for half, (pg, pc) in ((0, (pg0, pc0)), (1, (pg1, pc1))):
                        sl = slice(2 * half, 2 * half + 2)
                        for s in range(2):
                            o = 2 * half + s
                            nc.vector.tensor_mul(tg[:, o, :], pg[:, s, :], psbS[:])
                            nc.vector.tensor_mul(tc_[:, o, :], pc[:, s, :], psbS[:])
                        nc.scalar.activation(z[:, sl, :], tg[:, sl, :], AF.Sigmoid)
                        nc.vector.tensor_scalar(
                            zb[:, sl, :], z[:, sl, :], -1.0, 1.0, OP.mult, OP.add
                        )
                        nc.vector.tensor_mul(bsc[:, sl, :], tc_[:, sl, :], z[:, sl, :])
                        for s in range(2):
                            o = 2 * half + s
                            init = 0.0 if i == 0 else h_prev[o][:, CHUNK - 1 : CHUNK]
                            nc.vector.tensor_tensor_scan(
                                h[:, o, :], zb[:, o, :], bsc[:, o, :], init,
                                OP.mult, OP.add,
                            )
                            h_prev[o] = h[:, o, :]
                        if half == 0:
                            if i + 3 < N_CHUNKS:
                                stg_copy(i + 3)
                            if i % 2 == 0 and i + 2 < N_CHUNKS:
                                newton((i + 2) // 2)
                    ot = wpool.tile([128, OT, CHUNK], FP16, tag="ot")
                    nc.gpsimd.tensor_add(ot[:], h[:], xr_t[i][:])
                nc.gpsimd.dma_start(out_d[i].transpose([1, 0, 2]), ot[:])

            # ---- prologue: chunk-0 mains keep the PE hot while the stats
            # chain (squares/stats/newton) for chunks 0-2 runs on ACT/GpSimd ----
            load_x(0, split=True)
            load_consts()
            load_xr(0)
            load_x(1)
            load_x(2)
            pre0 = gemm_main(0, 0)
            pre1 = gemm_main(0, 1)
            squares(0)
            squares(1)
            squares(2)
            stats(0)
            stats(1)
            stats(2)
            stg_copy(0)
            stg_copy(1)
            newton(0)
            stg_copy(2)
            chunk_body(0, pre=(pre0, pre1))
            for i in range(1, N_CHUNKS):
                chunk_body(i)

    nc.compile()
    return nc


def _prep_weights(gamma, beta, Wg, bg, Wc, bc, ohalf):
    """Host-side weight folding for one output half (fp8, 32x scaled)."""
    o0 = ohalf * OH
    perm = np.roll(np.arange(H), -o0)
    out = {}
    for nm, W, b in (("g", Wg, bg), ("c", Wc, bc)):
        W_h = W[o0 : o0 + OH]                                   # [OH, H]
        w_eff = ((W_h * gamma[None, :]).T)[perm]                # [H, OH]
        b_eff = b[o0 : o0 + OH] + W_h @ beta                    # [OH]
        w8 = (32.0 * w_eff).astype(F8)                          # [H, OH] fp8
        wsumq = w8.astype(np.float32).sum(axis=0) / 32.0        # [OH]
        aug = np.zeros((1, 2, OH), dtype=F8)
        aug[0, 0] = (-4.0 * wsumq).astype(F8)
        aug[0, 1] = (32.0 * b_eff).astype(F8)
        out["w" + nm] = np.ascontiguousarray(
            w8.reshape(KT // 2, 2, 128, OT, 128).transpose(2, 0, 3, 1, 4)
        )
        out["aug" + nm] = np.ascontiguousarray(
            aug.reshape(1, 2, OT, 128).transpose(0, 2, 1, 3)
        )
    onx = np.zeros((128, 2, 128), dtype=F8)
    onx[:, :, 0] = 1.0
    onq = np.zeros((128, 2, 128), dtype=F8)
    onq[:, :, 1] = 1.0
    out["onx"] = onx
    out["onq"] = onq
    out["onr"] = np.full((1, 128), 1.0 / 32.0, dtype=BF)
    return out


def kernel(x, gamma, beta, Wg, bg, Wc, bc):
    x = np.asarray(x, dtype=np.float32)
    gamma = np.asarray(gamma, dtype=np.float32)
    beta = np.asarray(beta, dtype=np.float32)
    Wg = np.asarray(Wg, dtype=np.float32)
    bg = np.asarray(bg, dtype=np.float32)
    Wc = np.asarray(Wc, dtype=np.float32)
    bc = np.asarray(bc, dtype=np.float32)

    if "nc" not in _CACHE:
        _CACHE["nc"] = _build()
    nc = _CACHE["nc"]

    xT = [np.ascontiguousarray(x[b].T) for b in range(B)]  # [H, T] each
    halves = [_prep_weights(gamma, beta, Wg, bg, Wc, bc, p) for p in range(2)]

    in_maps = []
    for c in range(N_CORES):
        b, p = divmod(c, 2)
        m = dict(halves[p])
        xr = xT[b] if p == 0 else np.roll(xT[b], -OH, axis=0)
        m["x8"] = np.ascontiguousarray(
            xr.astype(F8).reshape(KT, 128, N_CHUNKS, CHUNK).transpose(2, 1, 0, 3)
        )
        m["xr"] = np.ascontiguousarray(
            xr[:OH].astype(np.float16).reshape(OT, 128, N_CHUNKS, CHUNK).transpose(2, 1, 0, 3)
        )
        in_maps.append(m)

    trace = bool(int(os.environ.get("MINGRU_TRACE", "0")))
    kwargs = {}
    if trace:
        tmpdir = os.environ.get("MINGRU_TRACE_DIR") or None
        kwargs = dict(trace=True, tmpdir=tmpdir)
    res = run_bass_kernel_spmd(nc, in_maps, core_ids=list(range(N_CORES)), **kwargs)
    if trace:
        _CACHE["last_results"] = res

    out = np.empty((B, T, H), dtype=np.float32)
    for c in range(N_CORES):
        b, p = divmod(c, 2)
        oT = res.results[c]["outT"].astype(np.float32).transpose(1, 2, 0, 3).reshape(OH, T)
        out[b, :, p * OH : (p + 1) * OH] = oT.T
    return out


# revision 20
# speedup vs baseline: 1.0590x; 1.0590x over previous
"""MinGRU layer (LN -> gate/candidate Linear -> minGRU scan -> residual) on 8 trn2 cores.

Problem (hardcoded): x [B=4, T=4096, H=1024] fp32, weights Wg/Wc [1024,1024],
biases bg/bc [1024], LN gamma/beta [1024].

Sharding: core c = (batch b = c//2, output-half p = c%2). Each core receives the
full transposed batch row (H on partitions, T on free) and computes z/c for its
512 output channels over all T; the minGRU recurrence is elementwise over (b,h)
so no collectives are needed.

Per 512-col chunk, built around fp8 DoubleRow matmuls (each streams a PAIR of
k-tiles -> 2x bf16 GEMM throughput; weights pre-scaled by 32 on host so fp8
stays in normal range):
  1. GEMM on RAW fp8 x (no prescale): P' = 32W.x8 over 4 k-pair DR matmuls plus
     one rank-1 aug DR matmul carrying both LayerNorm corrections:
     slot0 = (-4*wsumq) x (8*mu), slot1 = (32*b_eff) x (1/rstd).  Then
     tmp = P' * (rstd/32 broadcast) = W.((x-mu)*rstd) + b exactly (post-scale
     keeps P' in fp32 PSUM - no fp8 requantize of the scaled activations).
  2. Stats by the same DR trick: ones-weights (col 0 / col 1) sum x8 and x8^2
     pairs into rows 0/1 of one PSUM bank; squares x8^2 on ScalarE.
  3. rstd via 2 Newton steps for 1/sqrt(var+eps) on GpSimd in a DMA-transposed
     [128, 2, 4] layout (t on partitions, both chunks of a pair at once) - no
     Ln/Exp, so the ACT table never leaves the sigmoid/square/copy set.
  4. z = sigmoid(tmp_g) on ScalarE (bias pre-folded via aug slot1); zbar = 1-z
     (VectorE 4x); bsc = tmp_c*z (GpSimd); h = scan(zbar, bsc) (VectorE, the
     only engine with tensor_tensor_scan); residual h + x(fp16) on VectorE in
     2x mode; fp16 output, host transposes back.
"""

import os
import numpy as np
import ml_dtypes

import concourse.bass as bass
import concourse.bacc as bacc
import concourse.tile as tile
from concourse import mybir
from concourse.bass_utils import run_bass_kernel_spmd

B, T, H = 4, 4096, 1024
EPS = 1e-5
N_CORES = 8
OH = H // 2          # output channels per core
CHUNK = 512
N_CHUNKS = T // CHUNK
KT = H // 128        # k-tiles (contraction)
OT = OH // 128       # o-tiles per core
NPAIR = N_CHUNKS // 2

F32 = mybir.dt.float32
BF16 = mybir.dt.bfloat16
FP16 = mybir.dt.float16
FP8 = mybir.dt.float8e4
AF = mybir.ActivationFunctionType
OP = mybir.AluOpType
DR = mybir.MatmulPerfMode.DoubleRow
BF = ml_dtypes.bfloat16
F8 = ml_dtypes.float8_e4m3

_CACHE = {}


def _build():
    nc = bacc.Bacc("TRN2", target_bir_lowering=False, debug=False)

    x8_d = nc.dram_tensor("x8", [N_CHUNKS, 128, KT, CHUNK], FP8, kind="ExternalInput").ap()
    xr_d = nc.dram_tensor("xr", [N_CHUNKS, 128, OT, CHUNK], FP16, kind="ExternalInput").ap()
    wg_d = nc.dram_tensor("wg", [128, KT // 2, OT, 2, 128], FP8, kind="ExternalInput").ap()
    wc_d = nc.dram_tensor("wc", [128, KT // 2, OT, 2, 128], FP8, kind="ExternalInput").ap()
    augg_d = nc.dram_tensor("augg", [1, OT, 2, 128], FP8, kind="ExternalInput").ap()
    augc_d = nc.dram_tensor("augc", [1, OT, 2, 128], FP8, kind="ExternalInput").ap()
    onx_d = nc.dram_tensor("onx", [128, 2, 128], FP8, kind="ExternalInput").ap()
    onq_d = nc.dram_tensor("onq", [128, 2, 128], FP8, kind="ExternalInput").ap()
    onr_d = nc.dram_tensor("onr", [1, 128], BF16, kind="ExternalInput").ap()
    out_d = nc.dram_tensor("outT", [N_CHUNKS, OT, 128, CHUNK], FP16, kind="ExternalOutput").ap()

    # Newton scratch in DRAM (partition-crossing transposes go through HBM).
    # stg rows per pair: (sx0, sq0, sx1, sq1)
    st_dram = nc.dram_tensor("st_sc", [NPAIR, 4, CHUNK], F32, kind="Internal").ap()
    rs_dram = nc.dram_tensor("rs_sc", [NPAIR, 2, CHUNK], BF16, kind="Internal").ap()
    ag_dram = nc.dram_tensor("ag_sc", [NPAIR, 2, 2, CHUNK], FP8, kind="Internal").ap()

    with tile.TileContext(nc) as tc:
        with (
            tc.tile_pool(name="const", bufs=1) as cpool,
            tc.tile_pool(name="xin", bufs=5) as xpool,
            tc.tile_pool(name="sq", bufs=5) as sqpool,
            tc.tile_pool(name="xr", bufs=3) as xrpool,
            tc.tile_pool(name="nt", bufs=2) as ntpool,
            tc.tile_pool(name="row", bufs=4) as rpool,
            tc.tile_pool(name="work", bufs=2) as wpool,
            tc.tile_pool(name="hbuf", bufs=2) as hpool,
            tc.tile_pool(name="psG", bufs=2, space="PSUM") as psG,
            tc.tile_pool(name="psC", bufs=1, space="PSUM") as psC,
            tc.tile_pool(name="psS", bufs=1, space="PSUM") as psS,
            tc.tile_pool(name="psb", bufs=1, space="PSUM") as psbp,
        ):
            wg_sb = cpool.tile([128, KT // 2, OT, 2, 128], FP8, tag="wg")
            wc_sb = cpool.tile([128, KT // 2, OT, 2, 128], FP8, tag="wc")
            augg = cpool.tile([1, OT, 2, 128], FP8, tag="augg")
            augc = cpool.tile([1, OT, 2, 128], FP8, tag="augc")
            onx = cpool.tile([128, 2, 128], FP8, tag="onx")
            onq = cpool.tile([128, 2, 128], FP8, tag="onq")
            onr = cpool.tile([1, 128], BF16, tag="onr")

            def load_consts():
                nc.scalar.dma_start(onx[:], onx_d)
                nc.scalar.dma_start(onq[:], onq_d)
                nc.scalar.dma_start(onr[:], onr_d)
                nc.scalar.dma_start(wg_sb[:], wg_d)
                nc.scalar.dma_start(wc_sb[:], wc_d)
                nc.scalar.dma_start(augg[:], augg_d)
                nc.scalar.dma_start(augc[:], augc_d)

            x8_t = [None] * N_CHUNKS
            xsq_t = [None] * N_CHUNKS
            xr_t = [None] * N_CHUNKS
            st_t = [None] * N_CHUNKS     # PSUM stats tiles
            rstd_t = [None] * N_CHUNKS   # [1,512] bf16 rows
            augr_t = [None] * N_CHUNKS   # [1,2,512] fp8 rows
            h_prev = [None] * OT

            def load_x(i, split=False):
                xc = xpool.tile([128, KT, CHUNK], FP8, tag="x8")
                if split:
                    half = KT // 2
                    nc.sync.dma_start(xc[:, :half, :], x8_d[i, :, :half, :])
                    nc.sync.dma_start(xc[:, half:, :], x8_d[i, :, half:, :])
                else:
                    nc.sync.dma_start(xc[:], x8_d[i])
                x8_t[i] = xc

            def load_xr(i):
                xr = xrpool.tile([128, OT, CHUNK], FP16, tag="xr")
                nc.sync.dma_start(xr[:], xr_d[i])
                xr_t[i] = xr

            def squares(i):
                xc = x8_t[i]
                sq = sqpool.tile([128, KT, CHUNK], FP8, tag="xsq")
                with nc.allow_low_precision(reason="fp8 squares only feed the var sum"):
                    half = KT // 2
                    nc.scalar.activation(sq[:, :half, :], xc[:, :half, :], AF.Square)
                    nc.scalar.activation(sq[:, half:, :], xc[:, half:, :], AF.Square)
                xsq_t[i] = sq

            def stats(i):
                """st row0 = sum_h x8, row1 = sum_h x8^2 (one PSUM bank)."""
                xc, sq = x8_t[i], xsq_t[i]
                st = psS.tile([128, CHUNK], F32, tag="st")
                for kp in range(KT // 2):
                    nc.tensor.matmul(
                        st[:], onx[:], xc[:, 2 * kp : 2 * kp + 2, :],
                        start=(kp == 0), stop=False, perf_mode=DR,
                    )
                for kp in range(KT // 2):
                    nc.tensor.matmul(
                        st[:], onq[:], sq[:, 2 * kp : 2 * kp + 2, :],
                        start=False, stop=(kp == KT // 2 - 1), perf_mode=DR,
                    )
                st_t[i] = st

            def stg_copy(i):
                # PSUM stats rows -> SBUF staging -> DRAM scratch
                pair, par = divmod(i, 2)
                stg = ntpool.tile([2, CHUNK], F32, tag="stg")
                nc.scalar.activation(stg[:], st_t[i][0:2, :], AF.Copy)
                nc.sync.dma_start(st_dram[pair, 2 * par : 2 * par + 2], stg[:])

            def newton(pair):
                """rstd = 1/sqrt(var+eps) via 2 Newton steps, t-on-partitions."""
                # [4,512] rows (a=chunk, two=kind) -> two loads of [128, a, 4]
                src3 = st_dram[pair].rearrange("(a two) (p j) -> two p a j", two=2, p=128)
                sxT = ntpool.tile([128, 2, 4], F32, tag="sxT")
                sqT = ntpool.tile([128, 2, 4], F32, tag="sqT")
                nc.sync.dma_start(sxT[:], src3[0])
                nc.sync.dma_start(sqT[:], src3[1])
                sx = sxT[:]
                sq = sqT[:]
                mu = ntpool.tile([128, 2, 4], F32, tag="mu")
                m2 = ntpool.tile([128, 2, 4], F32, tag="m2")
                v = ntpool.tile([128, 2, 4], F32, tag="v")
                y = ntpool.tile([128, 2, 4], F32, tag="y")
                t1 = ntpool.tile([128, 2, 4], F32, tag="t1")
                g = nc.gpsimd
                g.tensor_scalar_mul(mu[:], sx, 1.0 / H)
                g.tensor_mul(m2[:], mu[:], mu[:])         # mu^2
                g.tensor_scalar(t1[:], sq, 1.0 / H, EPS, OP.mult, OP.add)
                g.tensor_sub(v[:], t1[:], m2[:])          # var + eps
                g.tensor_scalar(y[:], v[:], -0.5, 1.5, OP.mult, OP.add)
                for _ in range(2):
                    g.tensor_mul(t1[:], v[:], y[:])
                    g.tensor_mul(t1[:], t1[:], y[:])
                    g.tensor_scalar(t1[:], t1[:], -0.5, 1.5, OP.mult, OP.add)
                    g.tensor_mul(y[:], y[:], t1[:])
                rT = ntpool.tile([128, 2, 4], BF16, tag="rT")
                m8 = ntpool.tile([128, 2, 4], FP8, tag="m8")
                i8 = ntpool.tile([128, 2, 4], FP8, tag="i8")
                with nc.allow_low_precision(reason="rstd bf16 / aug rows fp8"):
                    g.tensor_scalar_mul(rT[:], y[:], 1.0)
                    g.tensor_scalar_mul(m8[:], mu[:], 8.0)
                    g.tensor_mul(i8[:], v[:], y[:])       # 1/rstd = (var+eps)*rstd
                nc.sync.dma_start(
                    rs_dram[pair].rearrange("a (p j) -> p a j", p=128), rT[:]
                )
                agr = ag_dram[pair].rearrange("a s (p j) -> s p a j", p=128)
                nc.sync.dma_start(agr[0], m8[:])
                nc.sync.dma_start(agr[1], i8[:])
                for par in range(2):
                    i = 2 * pair + par
                    rr = rpool.tile([1, CHUNK], BF16, tag="rstd")
                    nc.sync.dma_start(rr[:], rs_dram[pair, par : par + 1, :])
                    rstd_t[i] = rr
                    ar = rpool.tile([1, 2, CHUNK], FP8, tag="augr")
                    nc.sync.dma_start(ar[:], ag_dram[pair, par].unsqueeze(0))
                    augr_t[i] = ar

            def gemm_main(i, half):
                """The 16 rstd-independent k-pair matmuls of one o-half."""
                xc = x8_t[i]
                pg = psG.tile([128, 2, CHUNK], F32, tag="pg")
                pc = psC.tile([128, 2, CHUNK], F32, tag="pc")
                for w_sb, dst in ((wg_sb, pg), (wc_sb, pc)):
                    for s in range(2):
                        o = 2 * half + s
                        for kp in range(KT // 2):
                            nc.tensor.matmul(
                                dst[:, s, :],
                                w_sb[:, kp, o],
                                xc[:, 2 * kp : 2 * kp + 2, :],
                                start=(kp == 0), stop=False, perf_mode=DR,
                                skip_group_check=True,
                            )
                return pg, pc

            def gemm_aug(i, half, pg, pc):
                """Rank-1 LN-correction matmuls (need mu/invrstd rows)."""
                ar = augr_t[i]
                for aug, dst in ((augg, pg), (augc, pc)):
                    for s in range(2):
                        o = 2 * half + s
                        nc.tensor.matmul(
                            dst[:, s, :], aug[:, o], ar[:],
                            start=False, stop=True, perf_mode=DR,
                            skip_group_check=True,
                        )

            def chunk_body(i, pre=None):
                if i + 3 < N_CHUNKS:
                    load_x(i + 3)
                if i + 1 < N_CHUNKS:
                    load_xr(i + 1)

                pg0, pc0 = pre[0] if pre else gemm_main(i, 0)
                # rstd-dependent PE work sits after 16 free-running matmuls
                psb = psbp.tile([128, CHUNK], F32, tag="psb")
                nc.tensor.matmul(psb[:], onr[:], rstd_t[i][:], start=True, stop=True)
                gemm_aug(i, 0, pg0, pc0)
                psbS = wpool.tile([128, CHUNK], BF16, tag="psbS")
                with nc.allow_low_precision(reason="bf16 rstd broadcast"):
                    nc.scalar.activation(psbS[:], psb[:], AF.Copy)
                if i + 3 < N_CHUNKS:
                    squares(i + 3)      # ACT fills while PE runs GEMMs
                pg1, pc1 = pre[1] if pre else gemm_main(i, 1)
                if i + 3 < N_CHUNKS:
                    stats(i + 3)        # PE mid-chunk
                gemm_aug(i, 1, pg1, pc1)

                with nc.allow_low_precision(reason="bf16 gate/candidate path"):
                    tg = wpool.tile([128, OT, CHUNK], BF16, tag="tg")
                    tc_ = wpool.tile([128, OT, CHUNK], BF16, tag="tcn")
                    z = wpool.tile([128, OT, CHUNK], BF16, tag="z")
                    zb = wpool.tile([128, OT, CHUNK], BF16, tag="zb")
                    bsc = wpool.tile([128, OT, CHUNK], BF16, tag="bsc")
                    h = hpool.tile([128, OT, CHUNK], BF16, tag="h")
                    for half, (pg, pc) in ((0, (pg0, pc0)), (1, (pg1, pc1))):
                        sl = slice(2 * half, 2 * half + 2)
                        for s in range(2):
                            o = 2 * half + s
                            nc.vector.tensor_mul(tg[:, o, :], pg[:, s, :], psbS[:])
                            nc.vector.tensor_mul(tc_[:, o, :], pc[:, s, :], psbS[:])
                        nc.scalar.activation(z[:, sl, :], tg[:, sl, :], AF.Sigmoid)
                        nc.vector.tensor_scalar(
                            zb[:, sl, :], z[:, sl, :], -1.0, 1.0, OP.mult, OP.add
                        )
                        nc.vector.tensor_mul(bsc[:, sl, :], tc_[:, sl, :], z[:, sl, :])
                        for s in range(2):
                            o = 2 * half + s
                            init = 0.0 if i == 0 else h_prev[o][:, CHUNK - 1 : CHUNK]
                            nc.vector.tensor_tensor_scan(
                                h[:, o, :], zb[:, o, :], bsc[:, o, :], init,
                                OP.mult, OP.add,
                            )
                            h_prev[o] = h[:, o, :]
                        if half == 0:
                            if i + 3 < N_CHUNKS:
                                stg_copy(i + 3)
                            if i % 2 == 0 and i + 2 < N_CHUNKS:
                                newton((i + 2) // 2)
                    ot = wpool.tile([128, OT, CHUNK], FP16, tag="ot")
                    nc.gpsimd.tensor_add(ot[:], h[:], xr_t[i][:])
                nc.gpsimd.dma_start(out_d[i].transpose([1, 0, 2]), ot[:])

            # ---- prologue: chunk-0 mains keep the PE hot while the stats
            # chain (squares/stats/newton) for chunks 0-2 runs on ACT/GpSimd ----
            load_x(0, split=True)
            load_consts()
            load_xr(0)
            load_x(1)
            load_x(2)
            pre0 = gemm_main(0, 0)
            pre1 = gemm_main(0, 1)
            squares(0)
            squares(1)
            squares(2)
            stats(0)
            stats(1)
            stats(2)
            stg_copy(0)
            stg_copy(1)
            newton(0)
            stg_copy(2)
            chunk_body(0, pre=(pre0, pre1))
            for i in range(1, N_CHUNKS):
                chunk_body(i)

    nc.compile()
    return nc


def _prep_weights(gamma, beta, Wg, bg, Wc, bc, ohalf):
    """Host-side weight folding for one output half (fp8, 32x scaled)."""
    o0 = ohalf * OH
    perm = np.roll(np.arange(H), -o0)
    out = {}
    for nm, W, b in (("g", Wg, bg), ("c", Wc, bc)):
        W_h = W[o0 : o0 + OH]                                   # [OH, H]
        w_eff = ((W_h * gamma[None, :]).T)[perm]                # [H, OH]
        b_eff = b[o0 : o0 + OH] + W_h @ beta                    # [OH]
        w8 = (32.0 * w_eff).astype(F8)                          # [H, OH] fp8
        wsumq = w8.astype(np.float32).sum(axis=0) / 32.0        # [OH]
        aug = np.zeros((1, 2, OH), dtype=F8)
        aug[0, 0] = (-4.0 * wsumq).astype(F8)
        aug[0, 1] = (32.0 * b_eff).astype(F8)
        out["w" + nm] = np.ascontiguousarray(
            w8.reshape(KT // 2, 2, 128, OT, 128).transpose(2, 0, 3, 1, 4)
        )
        out["aug" + nm] = np.ascontiguousarray(
            aug.reshape(1, 2, OT, 128).transpose(0, 2, 1, 3)
        )
    onx = np.zeros((128, 2, 128), dtype=F8)
    onx[:, :, 0] = 1.0
    onq = np.zeros((128, 2, 128), dtype=F8)
    onq[:, :, 1] = 1.0
    out["onx"] = onx
    out["onq"] = onq
    out["onr"] = np.full((1, 128), 1.0 / 32.0, dtype=BF)
    return out


def kernel(x, gamma, beta, Wg, bg, Wc, bc):
    x = np.asarray(x, dtype=np.float32)
    gamma = np.asarray(gamma, dtype=np.float32)
    beta = np.asarray(beta, dtype=np.float32)
    Wg = np.asarray(Wg, dtype=np.float32)
    bg = np.asarray(bg, dtype=np.float32)
    Wc = np.asarray(Wc, dtype=np.float32)
    bc = np.asarray(bc, dtype=np.float32)

    if "nc" not in _CACHE:
        _CACHE["nc"] = _build()
    nc = _CACHE["nc"]

    xT = [np.ascontiguousarray(x[b].T) for b in range(B)]  # [H, T] each
    halves = [_prep_weights(gamma, beta, Wg, bg, Wc, bc, p) for p in range(2)]

    in_maps = []
    for c in range(N_CORES):
        b, p = divmod(c, 2)
        m = dict(halves[p])
        xr = xT[b] if p == 0 else np.roll(xT[b], -OH, axis=0)
        m["x8"] = np.ascontiguousarray(
            xr.astype(F8).reshape(KT, 128, N_CHUNKS, CHUNK).transpose(2, 1, 0, 3)
        )
        m["xr"] = np.ascontiguousarray(
            xr[:OH].astype(np.float16).reshape(OT, 128, N_CHUNKS, CHUNK).transpose(2, 1, 0, 3)
        )
        in_maps.append(m)

    trace = bool(int(os.environ.get("MINGRU_TRACE", "0")))
    kwargs = {}
    if trace:
        tmpdir = os.environ.get("MINGRU_TRACE_DIR") or None
        kwargs = dict(trace=True, tmpdir=tmpdir)
    res = run_bass_kernel_spmd(nc, in_maps, core_ids=list(range(N_CORES)), **kwargs)
    if trace:
        _CACHE["last_results"] = res

    out = np.empty((B, T, H), dtype=np.float32)
    for c in range(N_CORES):
        b, p = divmod(c, 2)
        oT = res.results[c]["outT"].astype(np.float32).transpose(1, 2, 0, 3).reshape(OH, T)
        out[b, :, p * OH : (p + 1) * OH] = oT.T
    return out


# revision 21
# speedup vs baseline: 1.1145x; 1.0523x over previous
"""MinGRU layer (LN -> gate/candidate Linear -> minGRU scan -> residual) on 8 trn2 cores.

Problem (hardcoded): x [B=4, T=4096, H=1024] fp32, weights Wg/Wc [1024,1024],
biases bg/bc [1024], LN gamma/beta [1024].

Sharding: core c = (batch b = c//2, output-half p = c%2). Every core receives
the full transposed batch row xT[b] = x[b].T (H on partitions, T on free) and
computes z/c for its 512 output channels over all T. The minGRU recurrence is
elementwise over (b, h), so with output-channel sharding each core scans its
own channels over the full sequence - no cross-core dependency, no collectives.

Per-core pipeline (layouts [h or o on partitions, t on free], 512-col chunks,
stats for chunk i+1 software-pipelined under the GEMMs of chunk i):
  1. LN folded algebraically: gate_pre[o,t] = sum_h W'[o,h]*(x[h,t]*rstd[t])
     - (mu*rstd)[t]*wsum[o] + b_eff[o], gamma/beta folded into W'/b_eff on
     host. mu/var from ones-matmuls on PE; x*rstd pre-scaled on VectorE in
     bf16 2x mode; the -mu*rstd*wsum term is a K=1 matmul row into the same
     PSUM tile; sigmoids read PSUM directly with per-partition bias.
  2. GEMMs in bf16 (fp32 PSUM). fp32/fp32r would force a non-overlapped
     ~187ns LDWEIGHTS per matmul; bf16 hides the weight load.
  3. rstd = exp(-0.5*ln(var+eps)) on ScalarE (vector.reciprocal is an 8x
     iterative divide; Rsqrt activation is banned for accuracy). Square/Copy/
     Sigmoid share one ACT table set; only Ln/Exp force 2 set switches/chunk.
  4. z = sigmoid(pre+bg); a = 1-z as sigmoid(-pre-bg) (independent of z);
     b = (c_pre+bc)*z as one scalar_tensor_tensor.
  5. h = tensor_tensor_scan(a, b) on VectorE, chained across chunks.
  6. out = h + x rows (fp32 residual input, separate from the bf16 GEMM x),
     on GpSimd; DMA out; host transposes shards back.
"""

import functools
import os
import numpy as np
import ml_dtypes

import concourse.bass as bass
import concourse.bacc as bacc
import concourse.tile as tile
import concourse.hw_specs as hw_specs
from concourse import mybir
from concourse.bass_utils import run_bass_kernel_spmd

# The table-load pass assigns each activation the FIRST act_func_set that
# contains it: Ln -> natural_log, Exp -> exp_and_others, costing two extra
# ~1.3us ACT_TABLE_LOADs per chunk. Strip ln/exp from those two sets (set
# indices stay aligned with act_info.json) so both resolve to the combined
# natural_log_exp_and_others set.
_orig_get_act_tables = hw_specs.get_activation_tables
_LN = mybir.ActivationFunctionType.Ln
_EXP = mybir.ActivationFunctionType.Exp


@functools.cache
def _patched_get_act_tables(module_arch):
    d = dict(_orig_get_act_tables(module_arch))
    for name in ("natural_log", "exp_and_others"):
        if name in d and "natural_log_exp_and_others" in d:
            d[name] = d[name] - {_LN, _EXP}
    return d


hw_specs.get_activation_tables = _patched_get_act_tables
bacc.get_activation_tables = _patched_get_act_tables

B, T, H = 4, 4096, 1024
EPS = 1e-5
N_CORES = 8
OH = H // 2          # output channels per core
CHUNK = 512
N_CHUNKS = T // CHUNK
KT = H // 128        # k-tiles (contraction)
OT = OH // 128       # o-tiles per core

F32 = mybir.dt.float32
BF16 = mybir.dt.bfloat16
AF = mybir.ActivationFunctionType
OP = mybir.AluOpType
BF = ml_dtypes.bfloat16

_CACHE = {}


def _build():
    nc = bacc.Bacc("TRN2", target_bir_lowering=False, debug=False)

    # all tensors host-pre-tiled so every DMA is fully contiguous
    xT_d = nc.dram_tensor("xT", [N_CHUNKS, 128, KT, CHUNK], BF16, kind="ExternalInput").ap()
    xr_d = nc.dram_tensor("xr", [N_CHUNKS, 128, OT, CHUNK], F32, kind="ExternalInput").ap()
    wg_d = nc.dram_tensor("wg", [128, KT, OH], BF16, kind="ExternalInput").ap()
    wc_d = nc.dram_tensor("wc", [128, KT, OH], BF16, kind="ExternalInput").ap()
    bg_d = nc.dram_tensor("bg", [128, OT], F32, kind="ExternalInput").ap()
    bgn_d = nc.dram_tensor("bgn", [128, OT], F32, kind="ExternalInput").ap()
    bc_d = nc.dram_tensor("bc", [128, OT], F32, kind="ExternalInput").ap()
    aug_g_d = nc.dram_tensor("aug_g", [1, OH], BF16, kind="ExternalInput").ap()
    aug_c_d = nc.dram_tensor("aug_c", [1, OH], BF16, kind="ExternalInput").ap()
    ones_d = nc.dram_tensor("ones", [128, 2], BF16, kind="ExternalInput").ap()
    onesr_d = nc.dram_tensor("onesr", [1, 128], BF16, kind="ExternalInput").ap()
    out_d = nc.dram_tensor("outT", [N_CHUNKS, OT, 128, CHUNK], F32, kind="ExternalOutput").ap()

    with tile.TileContext(nc) as tc:
        with (
            tc.tile_pool(name="const", bufs=1) as cpool,
            tc.tile_pool(name="xin", bufs=3) as xpool,
            tc.tile_pool(name="sq", bufs=2) as sqpool,
            tc.tile_pool(name="xnp", bufs=2) as xnpool,
            tc.tile_pool(name="stat", bufs=2) as spool,
            tc.tile_pool(name="work", bufs=3) as wpool,
            tc.tile_pool(name="hbuf", bufs=3) as hpool,
            tc.tile_pool(name="psA", bufs=3, space="PSUM") as psA,
            tc.tile_pool(name="psB", bufs=2, space="PSUM") as psB,
            tc.tile_pool(name="psS", bufs=2, space="PSUM") as psS,
            tc.tile_pool(name="psb", bufs=1, space="PSUM") as psbp,
        ):
            # ---- resident constants. The ones-vectors (needed by the first
            # stats matmul) go first on Sync; everything else rides the
            # Scalar queue so the first x chunk is not stuck behind it. ----
            ones_sb = cpool.tile([128, 2], BF16, tag="ones")
            nc.sync.dma_start(ones_sb[:], ones_d[:])
            onesA = ones_sb[:, 0:1]     # [128,1] lhsT for column sums
            onesR = cpool.tile([1, 128], BF16, tag="onesR")
            nc.sync.dma_start(onesR[:], onesr_d[:])
            wg_sb = cpool.tile([128, KT, OH], BF16, tag="wg")
            wc_sb = cpool.tile([128, KT, OH], BF16, tag="wc")
            bg_sb = cpool.tile([128, OT], F32, tag="bg")
            bgn_sb = cpool.tile([128, OT], F32, tag="bgn")
            bc_sb = cpool.tile([128, OT], F32, tag="bc")
            aug_g = cpool.tile([1, OH], BF16, tag="aug_g")
            aug_c = cpool.tile([1, OH], BF16, tag="aug_c")

            def load_consts():
                # emitted after the first x chunk so 2 MiB of weights don't
                # race it for HBM bandwidth at kernel start
                nc.scalar.dma_start(wg_sb[:], wg_d[:])
                nc.scalar.dma_start(wc_sb[:], wc_d[:])
                nc.scalar.dma_start(bg_sb[:], bg_d[:])
                nc.scalar.dma_start(bgn_sb[:], bgn_d[:])
                nc.scalar.dma_start(bc_sb[:], bc_d[:])
                nc.scalar.dma_start(aug_g[:], aug_g_d[:])
                nc.scalar.dma_start(aug_c[:], aug_c_d[:])

            h_prev = [None] * OT
            xc_t = [None] * N_CHUNKS     # raw bf16 x chunk
            xm_t = [None] * N_CHUNKS     # centered bf16 x chunk
            mu_t = [None] * N_CHUNKS
            rstd_t = [None] * N_CHUNKS

            def load_x(i, split=False):
                xc = xpool.tile([128, KT, CHUNK], BF16, tag="xc")
                src = xT_d[i]
                if split:  # let the first stats matmuls start on a half chunk
                    half = KT // 2
                    nc.sync.dma_start(xc[:, :half, :], src[:, :half, :])
                    nc.sync.dma_start(xc[:, half:, :], src[:, half:, :])
                else:
                    nc.sync.dma_start(xc[:], src)
                xc_t[i] = xc

            def stats_sumx(i):
                xc = xc_t[i]
                st = psS.tile([33, CHUNK], F32, tag="st")
                for k in range(KT):
                    nc.tensor.matmul(
                        st[0:1, :], onesA, xc[:, k, :],
                        start=(k == 0), stop=(k == KT - 1),
                    )
                mu = spool.tile([1, CHUNK], BF16, tag="mu")
                with nc.allow_low_precision(reason="bf16 mu for broadcast"):
                    nc.scalar.mul(mu[:], st[0:1, :], 1.0 / H)
                mu_t[i] = (st, mu)

            def stats_mid(i):
                """Square raw x (split ACT/GpSimd); mu^2 for the variance."""
                st, mu = mu_t[i]
                xc = xc_t[i]
                xsq = sqpool.tile([128, KT, CHUNK], BF16, tag="xsq")
                for k in range(KT):
                    if k < 4:
                        nc.scalar.activation(xsq[:, k, :], xc[:, k, :], AF.Square)
                    else:
                        nc.gpsimd.tensor_mul(xsq[:, k, :], xc[:, k, :], xc[:, k, :])
                mu2 = spool.tile([1, CHUNK], F32, tag="mu2")
                nc.scalar.activation(mu2[:], mu[:], AF.Square)
                mu_t[i] = (st, mu, xsq, mu2)

            def stats_sumsq(i):
                st, mu, xsq, mu2 = mu_t[i]
                for k in range(KT):
                    nc.tensor.matmul(
                        st[32:33, :], onesA, xsq[:, k, :],
                        start=(k == 0), stop=(k == KT - 1),
                    )

            def stats_tail(i):
                """var = E[x^2] - mu^2; rstd = exp(-0.5 ln(var+eps))."""
                st, mu, xsq, mu2 = mu_t[i]
                var = spool.tile([1, CHUNK], F32, tag="var")
                # var + eps = (E[x^2] + eps) - mu^2 with E[x^2] = st/H
                ex2 = spool.tile([1, CHUNK], F32, tag="ex2")
                nc.vector.tensor_scalar(
                    ex2[:], st[32:33, :], 1.0 / H, EPS, OP.mult, OP.add
                )
                nc.vector.tensor_sub(var[:], ex2[:], mu2[:])
                lnv = spool.tile([1, CHUNK], F32, tag="lnv")
                nc.scalar.activation(lnv[:], var[:], AF.Ln)
                rstd = spool.tile([1, CHUNK], BF16, tag="rstd")
                with nc.allow_low_precision(reason="bf16 rstd for bf16 GEMM prescale"):
                    nc.scalar.activation(rstd[:], lnv[:], AF.Exp, scale=-0.5)
                mr = spool.tile([1, CHUNK], BF16, tag="mr")
                nc.vector.tensor_mul(mr[:], mu[:], rstd[:])
                rstd_t[i] = (rstd, mr)

            def chunk_head(i):
                """Broadcast rstd, scale x."""
                rstd, mr = rstd_t[i]
                psb = psbp.tile([128, CHUNK], F32, tag="psbR")
                nc.tensor.matmul(psb[:], onesR[:], rstd[:], start=True, stop=True)
                rstdB = spool.tile([128, CHUNK], BF16, tag="rstdB")
                with nc.allow_low_precision(reason="bf16 rstd broadcast"):
                    nc.vector.tensor_scalar_mul(rstdB[:], psb[:], 1.0)
                xc = xc_t[i]
                xn = xnpool.tile([128, KT, CHUNK], BF16, tag="xn")
                for k in range(KT):
                    nc.vector.tensor_mul(xn[:, k, :], xc[:, k, :], rstdB[:])
                return xn

            def gemm_o(i, o, xn):
                og = o * 128
                mr = rstd_t[i][1]
                pg = psA.tile([128, CHUNK], F32, tag="pg")
                for k in range(KT):
                    nc.tensor.matmul(
                        pg[:], wg_sb[:, k, og : og + 128], xn[:, k, :],
                        start=(k == 0), stop=False,
                    )
                nc.tensor.matmul(
                    pg[:], aug_g[:, og : og + 128], mr[:], start=False, stop=True
                )
                pc = psB.tile([128, CHUNK], F32, tag="pc")
                for k in range(KT):
                    nc.tensor.matmul(
                        pc[:], wc_sb[:, k, og : og + 128], xn[:, k, :],
                        start=(k == 0), stop=False,
                    )
                nc.tensor.matmul(
                    pc[:], aug_c[:, og : og + 128], mr[:], start=False, stop=True
                )

                if o == 0:
                    xrc = xpool.tile([128, OT, CHUNK], F32, tag="xrc")
                    nc.sync.dma_start(xrc[:], xr_d[i])
                    xrc_t[0] = xrc
                xrc = xrc_t[0]

                z = wpool.tile([128, CHUNK], F32, tag="z")
                nc.scalar.activation(z[:], pg[:], AF.Sigmoid, bias=bg_sb[:, o : o + 1])
                # a = 1 - z = sigmoid(-(pre + bg)) -- independent of z
                a = wpool.tile([128, CHUNK], F32, tag="a")
                nc.scalar.activation(
                    a[:], pg[:], AF.Sigmoid, bias=bgn_sb[:, o : o + 1], scale=-1.0
                )
                bsc = wpool.tile([128, CHUNK], F32, tag="bsc")
                nc.vector.scalar_tensor_tensor(
                    bsc[:], pc[:], bc_sb[:, o : o + 1], z[:], OP.add, OP.mult
                )

                h = hpool.tile([128, CHUNK], F32, tag=f"h{o}")
                init = 0.0 if i == 0 else h_prev[o][:, CHUNK - 1 : CHUNK]
                nc.vector.tensor_tensor_scan(h[:], a[:], bsc[:], init, OP.mult, OP.add)
                h_prev[o] = h

                ot = wpool.tile([128, CHUNK], F32, tag="ot")
                nc.vector.tensor_add(ot[:], h[:], xrc[:, o, :])
                nc.sync.dma_start(out_d[i, o], ot[:])

            # ---- software pipeline: stats for i+1 run under the GEMMs of i,
            # interleaved so neither the PE queue nor the DVE queue waits ----
            xrc_t = [None]
            load_x(0, split=True)
            stats_sumx(0)
            load_consts()
            stats_mid(0)
            stats_sumsq(0)
            load_x(1)           # deepen startup: PE has stats(1) to chew on
            stats_sumx(1)       # while the chunk-0 rstd chain runs
            stats_tail(0)
            xn = chunk_head(0)
            for i in range(N_CHUNKS):
                nxt = i + 1 < N_CHUNKS
                if nxt and i > 0:
                    load_x(i + 1)
                    stats_sumx(i + 1)
                gemm_o(i, 0, xn)
                gemm_o(i, 1, xn)
                if nxt:
                    stats_mid(i + 1)
                gemm_o(i, 2, xn)
                if nxt:
                    stats_sumsq(i + 1)
                    stats_tail(i + 1)
                    xn_next = chunk_head(i + 1)
                gemm_o(i, 3, xn)
                if nxt:
                    xn = xn_next

    nc.compile()
    return nc


def _prep_inputs(gamma, beta, Wg, bg, Wc, bc, ohalf):
    """Host-side weight folding for one output half.

    The h-rows of the weights (and of xT, see kernel()) are rolled so this
    half's own output channels come first: the device residual then always
    reads x rows at k-tiles 0..OT-1 with one shared program across cores.
    """
    o0 = ohalf * OH
    perm = np.roll(np.arange(H), -o0)  # identity for half 0, swap halves for 1
    Wg_h = Wg[o0 : o0 + OH]          # [OH, H]
    Wc_h = Wc[o0 : o0 + OH]
    # lhsT layout [h, o], gamma folded into rows (h), rows permuted like xT
    wg_eff = ((Wg_h * gamma[None, :]).T)[perm].astype(np.float32)   # [H, OH]
    wc_eff = ((Wc_h * gamma[None, :]).T)[perm].astype(np.float32)
    bg_eff = (bg[o0 : o0 + OH] + Wg_h @ beta).astype(np.float32)
    bc_eff = (bc[o0 : o0 + OH] + Wc_h @ beta).astype(np.float32)
    wg_bf = wg_eff.astype(BF)
    wc_bf = wc_eff.astype(BF)
    wsum_g = wg_bf.astype(np.float32).sum(axis=0)
    wsum_c = wc_bf.astype(np.float32).sum(axis=0)

    def tile_w(w):  # [H, OH] -> [128, KT, OH]
        return np.ascontiguousarray(w.reshape(KT, 128, OH).transpose(1, 0, 2))

    return {
        "aug_g": np.ascontiguousarray(-wsum_g[None, :].astype(BF)),
        "aug_c": np.ascontiguousarray(-wsum_c[None, :].astype(BF)),
        "wg": tile_w(wg_bf),
        "wc": tile_w(wc_bf),
        "bg": np.ascontiguousarray(bg_eff.reshape(OT, 128).T),
        "bgn": np.ascontiguousarray(-bg_eff.reshape(OT, 128).T),
        "bc": np.ascontiguousarray(bc_eff.reshape(OT, 128).T),
        "ones": np.ones((128, 2), dtype=BF),
        "onesr": np.ones((1, 128), dtype=BF),
    }


def kernel(x, gamma, beta, Wg, bg, Wc, bc):
    x = np.asarray(x, dtype=np.float32)
    gamma = np.asarray(gamma, dtype=np.float32)
    beta = np.asarray(beta, dtype=np.float32)
    Wg = np.asarray(Wg, dtype=np.float32)
    bg = np.asarray(bg, dtype=np.float32)
    Wc = np.asarray(Wc, dtype=np.float32)
    bc = np.asarray(bc, dtype=np.float32)

    if "nc" not in _CACHE:
        _CACHE["nc"] = _build()
    nc = _CACHE["nc"]

    xT = [np.ascontiguousarray(x[b].T) for b in range(B)]  # [H, T] each
    halves = [_prep_inputs(gamma, beta, Wg, bg, Wc, bc, p) for p in range(2)]

    in_maps = []
    for c in range(N_CORES):
        b, p = divmod(c, 2)
        m = dict(halves[p])
        # roll h-rows to match the weight-row permutation for this half
        xr = xT[b] if p == 0 else np.roll(xT[b], -OH, axis=0)
        # pre-tile: [H, T] -> [chunks, 128, ktile, CHUNK] so DMAs are contiguous
        m["xT"] = np.ascontiguousarray(
            xr.astype(BF).reshape(KT, 128, N_CHUNKS, CHUNK).transpose(2, 1, 0, 3)
        )
        m["xr"] = np.ascontiguousarray(
            xr[:OH].reshape(OT, 128, N_CHUNKS, CHUNK).transpose(2, 1, 0, 3)
        )
        in_maps.append(m)

    trace = bool(int(os.environ.get("MINGRU_TRACE", "0")))
    kwargs = {}
    if trace:
        tmpdir = os.environ.get("MINGRU_TRACE_DIR") or None
        kwargs = dict(trace=True, tmpdir=tmpdir)
    res = run_bass_kernel_spmd(nc, in_maps, core_ids=list(range(N_CORES)), **kwargs)
    if trace:
        _CACHE["last_results"] = res

    out = np.empty((B, T, H), dtype=np.float32)
    for c in range(N_CORES):
        b, p = divmod(c, 2)
        # [chunks, OT, 128, CHUNK] -> [OH, T] -> [T, OH]
        oT = res.results[c]["outT"].transpose(1, 2, 0, 3).reshape(OH, T)
        out[b, :, p * OH : (p + 1) * OH] = oT.T
    return out

